# revision 7
# baseline (speedup 1.0000x reference)
"""Trainium2 Bass kernel for per-pixel dot-product attention.

Reference op (per pixel, over C=80 channels split q/k/v = 8/64/8):
    qk[v] = sum_k q[k] * K[k, v] / sqrt(8)
    attn  = softmax(qk over v)
    out[v] = attn[v] * V[v]

Data-parallel over 8 NeuronCores (core i: batch i//2, H-half i%2), bf16
end-to-end on device (inputs packed bf16 on host; ~0.5% quantization is
well inside the 2e-2 budget).

The per-pixel 1xK @ KxV matmul dominates.  Stock DVE ops need
64 mults + 56 tree-adds per pixel (120 elem-ops, 60 cyc/pixel-lane at
the bf16 2x tensor_tensor rate).  This kernel instead uses a CUSTOM DVE
micro-op, SEGSUM_MULT_ANT: a segmented multiply-scan that computes, per
contiguous 8-element page, the running sum of in0*in1 — so with K packed
(v, x, k) and q packed (x, k), the page-final element is the full 8-term
dot product qk[v, x].  Hand-written uop programs for both REGULAR and
2X_1PORT modes (3-state FSM: entry/steady/step, the step state resets
the accumulator at each SUB_DIM_DONE page boundary); perf_max=1 on the
instruction enables the 2x slot, verified engaged on HW (2292ns for
FD=4096 vs 4330ns at 1x).  At 2x the op consumes K at 2 elem/cyc/lane:
32 cyc/pixel-lane for the whole contraction — products, k-tree and the
final scale all collapse into one op class, and the k-sum accumulates in
fp32 internally (better precision than the old bf16 add-tree).  Only
odd output positions hold exact prefixes at 2x (even positions get the
pair-complete value); the consumer (ScalarE exp) reads only position 7
of each page via a stride-8 AP, which is exact.

ScalarE: e = exp(qk/sqrt 8) (strided read, free on ACT), r = exp(-ln s)
(both live in natural_log_exp_and_others — the Bacc subclass pins Exp
there to avoid per-chunk act-table thrash).  DVE softmax tail per
pixel: denominator pair-tree over v (3.5 cyc), e*V (4), r-broadcast
multiply (4).  Total DVE ~44 cyc/pixel-lane = ~47us busy: the kernel is
now DMA-bound (21 MB in + 2 MB out per core ~ 64.5us at 358 GB/s), so
chunks are sized for DMA pacing with a small last chunk to shorten the
post-stream tail.

Schedule: one-chunk software-pipeline skew as before (chunk j+1's
segsums run between chunk j's exp and its softmax tail), two-chunk skew
on the r-multiply so DVE never waits on the ACT ln/exp round trip.
V-loads are deferred one chunk on the sync ring; output stores ride the
scalar ring.
"""

import numpy as np
import ml_dtypes

NK = 8
NV = 8
C = NK + NK * NV + NV  # 80
B, H, W = 4, 512, 512
N_CORES = 8
ROWS = H // 2            # rows per core
PIX = ROWS * W           # pixels per core (131072)
COLS = PIX // 128        # columns per partition (1024)
_SCALE = 1.0 / float(np.sqrt(NK))
BF16 = ml_dtypes.bfloat16

CHUNK_COLS = [96, 192, 224, 224, 224, 64]


def _ensure_path():
    import sys
    p = "/opt/trn_rl_repo"
    if p not in sys.path:
        sys.path.insert(0, p)


def build_segsum_op():
    """Create + register the SEGSUM_MULT_ANT custom DVE op (idempotent).

    out[p, s, i] = sum_{j<=i} in0[p, s, j]*in1[p, s, j]   (page length 8)

    REGULAR program: blk0 p=src0*src1; blk1 A=CURR+p (steady) / A=p (step);
    carry to blk7; write WR0_LO.  2X_1PORT program: blk0 p0=SRC_0*SRC_1
    (stage HI operands on delay chains), blk1 p1=SRC_0_HI*SRC_1_HI (capture
    p0), blk2 c=p0+p1, blk3 A=CURR+c / c; write WR0_LO+WR0_HI=A (even
    positions approximate, odd exact).  FSM per mode: uop0 entry (reset,
    COUNT(1)->steady), uop1 steady (SUB_DIM_DONE->step), uop2 step (reset,
    COUNT(1)->steady)."""
    _ensure_path()
    from concourse import dve_ops
    from concourse.dve_spec import Spec, Src0, Src1, Bin, Scan, Zero
    from concourse.dve_spec import AluOp as SAluOp
    from concourse.dve_uop import (
        AluInp, AluOp, DelayInp, DveOpSpec, InpSel, OutPath, OutSel,
        Trigger, UopConfig, UopDpConfig, ENABLE,
    )

    NAME = "SEGSUM_MULT_ANT"
    for o in dve_ops.OPS:
        if o.name == NAME:
            return o

    def _ref(in0, in1, c0, c1, c2):
        p = in0.astype(np.float32) * in1.astype(np.float32)
        return np.cumsum(p, axis=-1).astype(np.float32)

    def _dp_regular(step):
        blks = [UopDpConfig() for _ in range(8)]
        blks[0].enable_alu(AluOp.MULTIPLY, AluInp.PREV_ALU_OUT, AluInp.PREV_DELAY_0)
        if step:
            blks[1].enable_alu(AluOp.BYPASS, AluInp.PREV_ALU_OUT)
        else:
            blks[1].enable_alu(AluOp.ADD, AluInp.CURR_ALU_OUT, AluInp.PREV_ALU_OUT)
        for b in range(2, 8):
            blks[b].pass_through_alu()
        return blks

    def _dp_2x(step):
        blks = [UopDpConfig() for _ in range(8)]
        blks[0].enable_alu(AluOp.MULTIPLY, AluInp.PREV_ALU_OUT, AluInp.PREV_DELAY_0)
        blks[0].enable_delay_from_src(DelayInp.PREV_DELAY, 1)  # chain1 <- lane2 (SRC_0_HI)
        blks[0].enable_delay_from_src(DelayInp.PREV_DELAY, 2)  # chain2 <- lane3 (SRC_1_HI)
        blks[1].enable_alu(AluOp.MULTIPLY, AluInp.PREV_DELAY_1, AluInp.PREV_DELAY_2)
        blks[1].enable_delay_from_src(DelayInp.PREV_ALU_OUT, 0)  # chain0 <- p0
        blks[2].enable_alu(AluOp.ADD, AluInp.PREV_ALU_OUT, AluInp.PREV_DELAY_0)
        if step:
            blks[3].enable_alu(AluOp.BYPASS, AluInp.PREV_ALU_OUT)
        else:
            blks[3].enable_alu(AluOp.ADD, AluInp.CURR_ALU_OUT, AluInp.PREV_ALU_OUT)
        for b in range(4, 8):
            blks[b].pass_through_alu()
        return blks

    def _uop(dp, two_x, entry):
        u = UopConfig()
        u.inp[0] = InpSel.SRC_0
        u.inp[1] = InpSel.SRC_1
        u.inp_enable[0] = ENABLE
        u.inp_enable[1] = ENABLE
        if two_x:
            u.inp[2] = InpSel.SRC_0_HI
            u.inp[3] = InpSel.SRC_1_HI
            u.inp_enable[2] = ENABLE
            u.inp_enable[3] = ENABLE
        u.out[OutPath.WR0_LO] = OutSel.ALU_OUT
        u.out_enable[OutPath.WR0_LO] = ENABLE
        if two_x:
            u.out[OutPath.WR0_HI] = OutSel.ALU_OUT
            u.out_enable[OutPath.WR0_HI] = ENABLE
        u.require_inp0 = ENABLE
        u.require_inp1 = ENABLE
        if entry is None:  # steady
            u.trigger = (Trigger.SRC_TENSOR_DONE, Trigger.SUB_DIM_DONE, Trigger.NONE)
            u.next_uop = (0, 2, 0)
        else:  # entry/step: reset pair, one cycle, then steady
            u.trigger = (Trigger.SRC_TENSOR_DONE, Trigger.SUB_DIM_DONE, Trigger.COUNT)
            u.next_uop = (0, 2, 1)
            u.repeat_count = 1
        u.datapath_config = dp
        return u

    uops_1x = [
        _uop(_dp_regular(True), False, True),
        _uop(_dp_regular(False), False, None),
        _uop(_dp_regular(True), False, False),
    ]
    uops_2x = [
        _uop(_dp_2x(True), True, True),
        _uop(_dp_2x(False), True, None),
        _uop(_dp_2x(True), True, False),
    ]

    class _SegsumOp(dve_ops.DveOp):
        def compile(self, ver):
            assert ver == "v3", f"SEGSUM_MULT_ANT authored for v3 only, got {ver}"
            spec = DveOpSpec(
                name=self.name,
                opcode=dve_ops.get_dve_sub_opcode(self.name),
                uops=uops_1x,
                uops_2x=uops_2x,
                perf_max=1,
                rd1_en=True,
            )
            spec.validate(ver)
            return spec

    spec = Spec(
        body=Scan(SAluOp.ADD, Bin(SAluOp.MULTIPLY, Src0, Src1), init=Zero),
        reference=_ref,
    )
    op = _SegsumOp(name=NAME, spec=spec, subdim=True, uops_sha={})
    dve_ops.OPS.append(op)
    dve_ops.CUSTOM_DVE_SPECS[NAME] = spec
    dve_ops._SUB_OPCODE_FOR_NAME[NAME] = (
        dve_ops._CUSTOM_DVE_ROW_BASE + len(dve_ops.OPS) - 1
    )
    return op


def emit_segsum(nc, op, out, in0, in1):
    """SEGSUM_MULT_ANT at 2x: APs [P, S, 8], unit inner stride, 4B-aligned."""
    from concourse import bass_isa, mybir
    from concourse.dve_ops import get_dve_sub_opcode

    v = nc.vector
    if op.name not in nc.m.ant_custom_dve_ops:
        nc.m.ant_custom_dve_ops = sorted({*nc.m.ant_custom_dve_ops, op.name})
    shape = bass_isa.CustomDveShape.STT
    isa_opcode = nc.isa.Opcode[
        f"NEURON_ISA_TPB_OPCODE_CUSTOM_DVE_ANT_{shape.slot()}"
    ].value
    ins = [
        v.lower_ap(in0, for_isa=True, opt=False),
        v.lower_ap(in1, for_isa=True, opt=False),
        mybir.ImmediateValue(dtype=mybir.dt.float32, value=0.0),
        mybir.ImmediateValue(dtype=mybir.dt.float32, value=0.0),
    ]
    outs = [v.lower_ap(out, for_isa=True, opt=False)]
    return v.add_instruction(
        bass_isa.InstCustomDveAnt(
            name=nc.get_next_instruction_name(),
            op_name=op.name,
            rd1_en=True,
            subdim=0x02,
            imm2=0.0,
            shape=shape,
            row=get_dve_sub_opcode(op.name),
            isa_opcode=isa_opcode,
            perf_max=1,
            ins=ins,
            outs=outs,
        )
    )


def build_nc(chunk_cols=None, recip_on_act=True):
    """Per-core Bass program over a packed (128, C*COLS) bf16 shard.

    Chunk j occupies columns [C*off, C*(off+ncol)) of x; per partition the
    chunk block is [q: (x,k)][K: (v,x,k)][V: (v,x)] so the segsum op reads
    unit-stride pages of 8 and q/K/V loads are single contiguous spans."""
    _ensure_path()
    import concourse.tile as tile
    from concourse import bacc, mybir

    f32 = mybir.dt.float32
    bf = mybir.dt.bfloat16
    if chunk_cols is None:
        chunk_cols = CHUNK_COLS
    assert sum(chunk_cols) == COLS

    import bass_rust as _bass_rust
    from concourse.hw_specs import get_activation_tables

    segsum_op = build_segsum_op()

    class _Bacc(bacc.Bacc):
        """Pin Exp to natural_log_exp_and_others (drop `exp` from the
        exp_and_others set) — one ACT_TABLE_LOAD, no per-chunk thrash."""

        def insert_act_table_loads(self):
            has_activation = any(
                isinstance(i, mybir.InstActivation)
                for b in self.main_func.blocks
                for i in b.instructions
            )
            if not has_activation:
                return
            exp_fn = mybir.ActivationFunctionType.Exp
            tables = []
            for name, funcs in get_activation_tables(self.m.arch).items():
                if name == "exp_and_others":
                    funcs = funcs - {exp_fn}
                tables.append((name, funcs))
            _bass_rust.insert_act_table_loads(self, tables)

    nc = _Bacc("TRN2", target_bir_lowering=False, debug=False)
    x = nc.dram_tensor("x", [128, C * COLS], bf, kind="ExternalInput")
    y = nc.dram_tensor("y", [128, NV * COLS], bf, kind="ExternalOutput")

    with tile.TileContext(nc) as tc:
        with (
            tc.tile_pool(name="inp", bufs=2) as in_pool,
            tc.tile_pool(name="work", bufs=2) as work_pool,
            tc.tile_pool(name="pipe", bufs=2) as pipe_pool,
        ):
            offs = []
            o = 0
            for ncol in chunk_cols:
                offs.append(o)
                o += ncol
            state = {}
            v_tiles = {}

            def emit_v_load(j):
                ncol, off = chunk_cols[j], offs[j]
                base = C * off
                nc.sync.dma_start(
                    out=v_tiles[j],
                    in_=x[:, base + (NK + NK * NV) * ncol:base + C * ncol])

            def emit_head(j):
                """DMA chunk j; 8 segsum dot-product scans; ACT exp."""
                ncol, off = chunk_cols[j], offs[j]
                base = C * off
                q_t = in_pool.tile([128, NK * ncol], bf, name=f"q{j}", tag="q",
                                   bufs=3)
                k_t = in_pool.tile([128, NK * NV * ncol], bf, name=f"k{j}",
                                   tag="k", bufs=3)
                v_t = in_pool.tile([128, NV * ncol], bf, name=f"v{j}",
                                   tag="v", bufs=3)
                v_tiles[j] = v_t
                nc.sync.dma_start(out=q_t, in_=x[:, base:base + NK * ncol])
                kbase = base + NK * ncol
                # K in two halves (v 0-3, v 4-7): the first 4 segsums start
                # on the first half — hides the straggler DMA queue's lag
                half = NK * NV * ncol // 2
                nc.sync.dma_start(out=k_t[:, :half],
                                  in_=x[:, kbase:kbase + half])
                nc.sync.dma_start(out=k_t[:, half:],
                                  in_=x[:, kbase + half:kbase + NK * NV * ncol])
                # V of the PREVIOUS chunk loads after this chunk's K (V is not
                # read until tailA(j-1), one pipeline stage later)
                if j > 0:
                    emit_v_load(j - 1)
                if j == len(chunk_cols) - 1:
                    emit_v_load(j)

                # qk[v, x] = segsum(K[v, x, :] * q[x, :])[..., 7]
                prod = work_pool.tile([128, NK * NV * ncol], bf,
                                      name=f"prod{j}", tag="prod")
                p4 = prod.rearrange("p (v s n) -> p v s n", v=NV, n=NK)
                k4 = k_t.rearrange("p (v s n) -> p v s n", v=NV, n=NK)
                q3 = q_t.rearrange("p (s n) -> p s n", n=NK)
                for vv in range(NV):
                    emit_segsum(nc, segsum_op, p4[:, vv], k4[:, vv], q3)

                # e = exp(qk / sqrt(NK)) on ScalarE, reading page-final
                # elements at stride 8 (exact positions of the 2x scan)
                e = pipe_pool.tile([128, NV * ncol], bf, name=f"e{j}", tag="e",
                                   bufs=3)
                qk = p4[:, :, :, NK - 1]
                e3 = e.rearrange("p (v s) -> p v s", v=NV)
                nc.scalar.activation(e3, qk,
                                     mybir.ActivationFunctionType.Exp,
                                     scale=_SCALE)
                state[j] = (e, v_t)

            def emit_tail_a(j):
                """Chunk j softmax part 1: denom pair-tree, r on ACT, e*V."""
                ncol, off = chunk_cols[j], offs[j]
                e, v_t = state[j]
                t1 = pipe_pool.tile([128, 4 * ncol], bf, name=f"t1{j}", tag="t1")
                with nc.allow_low_precision(reason="softmax denom in bf16"):
                    nc.vector.tensor_tensor(t1, e[:, 0:4 * ncol], e[:, 4 * ncol:],
                                            mybir.AluOpType.add)
                    nc.vector.tensor_tensor(t1[:, 0:2 * ncol], t1[:, 0:2 * ncol],
                                            t1[:, 2 * ncol:4 * ncol],
                                            mybir.AluOpType.add)
                    s = t1[:, 2 * ncol:3 * ncol]
                    nc.vector.tensor_tensor(s, t1[:, 0:ncol],
                                            t1[:, ncol:2 * ncol],
                                            mybir.AluOpType.add)
                r = pipe_pool.tile([128, ncol], bf, name=f"r{j}", tag="r")
                if recip_on_act:
                    # r = exp(-ln s) on ScalarE: off the DVE critical path,
                    # single act-table set (ln+exp coexist)
                    ls = pipe_pool.tile([128, ncol], f32, name=f"ls{j}", tag="ls")
                    nc.scalar.activation(ls, s, mybir.ActivationFunctionType.Ln)
                    nc.scalar.activation(r, ls, mybir.ActivationFunctionType.Exp,
                                         scale=-1.0)
                else:
                    rf = pipe_pool.tile([128, ncol], f32, name=f"rf{j}", tag="ls")
                    nc.vector.reciprocal(rf, s)
                    with nc.allow_low_precision(reason="r in bf16"):
                        nc.vector.tensor_copy(r, rf)

                # e *= V (does not depend on r; covers the ACT latency)
                e3 = e.rearrange("p (v x) -> p v x", v=NV)
                v3 = v_t.rearrange("p (v x) -> p v x", v=NV)
                nc.vector.tensor_tensor(e3, e3, v3, mybir.AluOpType.mult)
                state[j] = (e, r)

            def emit_tail_b(j):
                """Chunk j softmax part 2: multiply by r, store on the
                scalar ring."""
                ncol, off = chunk_cols[j], offs[j]
                e, r = state.pop(j)
                e3 = e.rearrange("p (v x) -> p v x", v=NV)
                r_b = r.unsqueeze(1).broadcast_to((128, NV, ncol))
                nc.vector.tensor_tensor(e3, r_b, e3, mybir.AluOpType.mult)
                nc.scalar.dma_start(out=y[:, NV * off:NV * (off + ncol)], in_=e)

            # software pipeline, two-chunk skew on the r-multiply:
            #   head(0), head(1), tailA(0), head(2), tailB(0), tailA(1), ...
            n = len(chunk_cols)
            for j in range(n):
                emit_head(j)
                if j >= 2:
                    emit_tail_b(j - 2)
                if j >= 1 and j != n - 1:
                    emit_tail_a(j - 1)
            emit_tail_a(n - 2)
            emit_tail_b(n - 2)
            emit_tail_a(n - 1)
            emit_tail_b(n - 1)
    nc.compile()
    return nc


_NC_CACHE = {}

BUILD_CFG = {
    "chunk_cols": CHUNK_COLS,
}


def _get_nc(**cfg):
    cfg = {**BUILD_CFG, **cfg}
    key = tuple(sorted(
        (k, tuple(v) if isinstance(v, list) else v) for k, v in cfg.items()
    ))
    if key not in _NC_CACHE:
        _NC_CACHE[key] = build_nc(**cfg)
    return _NC_CACHE[key]


def make_in_maps(inp, chunk_cols):
    """Shard + pack: core i gets batch i//2, H-half i%2, as a bf16
    (128, C*COLS) array laid out [p][chunk][q:(x,k)][K:(v,x,k)][V:(v,x)]."""
    in_maps = []
    for core in range(N_CORES):
        b, half = core // 2, core % 2
        shard = np.asarray(
            inp[b, :, half * ROWS:(half + 1) * ROWS, :], dtype=np.float32
        ).reshape(C, 128, COLS).astype(BF16)
        q_all = shard[:NK]                                   # (k, p, col)
        k_all = shard[NK:NK + NK * NV].reshape(NK, NV, 128, COLS)
        v_all = shard[NK + NK * NV:]                         # (v, p, col)
        blocks = []
        off = 0
        for ncol in chunk_cols:
            sl = slice(off, off + ncol)
            qb = np.ascontiguousarray(
                q_all[:, :, sl].transpose(1, 2, 0)).reshape(128, NK * ncol)
            kb = np.ascontiguousarray(
                k_all[:, :, :, sl].transpose(2, 1, 3, 0)).reshape(
                    128, NK * NV * ncol)
            vb = np.ascontiguousarray(
                v_all[:, :, sl].transpose(1, 0, 2)).reshape(128, NV * ncol)
            blocks.append(np.concatenate([qb, kb, vb], axis=1))
            off += ncol
        in_maps.append({"x": np.concatenate(blocks, axis=1)})
    return in_maps


def assemble_out(results, chunk_cols):
    out = np.empty((B, NV, H, W), np.float32)
    for core in range(N_CORES):
        b, half = core // 2, core % 2
        yb = results[core]["y"]  # (128, NV*COLS) bf16, chunk-major
        blocks = []
        off = 0
        for ncol in chunk_cols:
            blocks.append(
                yb[:, NV * off:NV * (off + ncol)].reshape(128, NV, ncol))
            off += ncol
        full = np.concatenate(blocks, axis=2)            # (128, NV, COLS)
        out[b, :, half * ROWS:(half + 1) * ROWS, :] = (
            full.transpose(1, 0, 2).astype(np.float32).reshape(NV, ROWS, W))
    return out


def run_spmd(inp, trace=False, build_cfg=None, **kwargs):
    """Run the SPMD kernel on 8 cores; returns (full_output, BassKernelResults)."""
    _ensure_path()
    from concourse.bass_utils import run_bass_kernel_spmd

    inp = np.asarray(inp)
    assert inp.shape == (B, C, H, W), inp.shape
    cfg = {**BUILD_CFG, **(build_cfg or {})}
    nc = _get_nc(**cfg)
    res = run_bass_kernel_spmd(
        nc, make_in_maps(inp, cfg["chunk_cols"]), list(range(N_CORES)),
        trace=trace, **kwargs
    )
    return assemble_out(res.results, cfg["chunk_cols"]), res


def kernel(inp):
    out, _ = run_spmd(inp, trace=False)
    return out


# revision 11
# speedup vs baseline: 1.0605x; 1.0605x over previous
"""Trainium2 Bass kernel for per-pixel dot-product attention.

Reference op (per pixel, over C=80 channels split q/k/v = 8/64/8):
    qk[v] = sum_k q[k] * K[k, v] / sqrt(8)
    attn  = softmax(qk over v)
    out[v] = attn[v] * V[v]

Data-parallel over 8 NeuronCores (core i: batch i//2, H-half i%2), bf16
end-to-end on device (inputs packed bf16 on host; ~0.5% quantization is
well inside the 2e-2 budget).

The per-pixel 1xK @ KxV matmul dominates.  Stock DVE ops need
64 mults + 56 tree-adds per pixel (120 elem-ops, 60 cyc/pixel-lane at
the bf16 2x tensor_tensor rate).  This kernel instead uses a CUSTOM DVE
micro-op, SEGSUM_MULT_ANT: a segmented multiply-scan that computes, per
contiguous 8-element page, the running sum of in0*in1 — so with K packed
(v, x, k) and q packed (x, k), the page-final element is the full 8-term
dot product qk[v, x].  Hand-written uop programs for both REGULAR and
2X_1PORT modes (3-state FSM: entry/steady/step, the step state resets
the accumulator at each SUB_DIM_DONE page boundary); perf_max=1 on the
instruction enables the 2x slot, verified engaged on HW (2292ns for
FD=4096 vs 4330ns at 1x).  At 2x the op consumes K at 2 elem/cyc/lane:
32 cyc/pixel-lane for the whole contraction — products, k-tree and the
final scale all collapse into one op class, and the k-sum accumulates in
fp32 internally (better precision than the old bf16 add-tree).  Only
odd output positions hold exact prefixes at 2x (even positions get the
pair-complete value); the consumer (ScalarE exp) reads only position 7
of each page via a stride-8 AP, which is exact.

ScalarE: e = exp(qk/sqrt 8) (strided read, free on ACT), r = exp(-ln s)
(both live in natural_log_exp_and_others — the Bacc subclass pins Exp
there to avoid per-chunk act-table thrash).  DVE softmax tail per
pixel: denominator pair-tree over v (3.5 cyc), e*V (4), r-broadcast
multiply (4).  Total DVE ~44 cyc/pixel-lane = ~47us busy: the kernel is
now DMA-bound (21 MB in + 2 MB out per core ~ 64.5us at 358 GB/s), so
chunks are sized for DMA pacing with a small last chunk to shorten the
post-stream tail.

Schedule: one-chunk software-pipeline skew as before (chunk j+1's
segsums run between chunk j's exp and its softmax tail), two-chunk skew
on the r-multiply so DVE never waits on the ACT ln/exp round trip.
V-loads are deferred one chunk on the sync ring; output stores ride the
scalar ring.
"""

import numpy as np
import ml_dtypes

NK = 8
NV = 8
C = NK + NK * NV + NV  # 80
B, H, W = 4, 512, 512
N_CORES = 8
ROWS = H // 2            # rows per core
PIX = ROWS * W           # pixels per core (131072)
COLS = PIX // 128        # columns per partition (1024)
_SCALE = 1.0 / float(np.sqrt(NK))
BF16 = ml_dtypes.bfloat16

CHUNK_COLS = [128, 224, 224, 224, 160, 64]


def _ensure_path():
    import sys
    p = "/opt/trn_rl_repo"
    if p not in sys.path:
        sys.path.insert(0, p)


def build_segsum_op():
    """Create + register the SEGSUM_MULT_ANT custom DVE op (idempotent).

    out[p, s, i] = sum_{j<=i} in0[p, s, j]*in1[p, s, j]   (page length 8)

    REGULAR program: blk0 p=src0*src1; blk1 A=CURR+p (steady) / A=p (step);
    carry to blk7; write WR0_LO.  2X_1PORT program: blk0 p0=SRC_0*SRC_1
    (stage HI operands on delay chains), blk1 p1=SRC_0_HI*SRC_1_HI (capture
    p0), blk2 c=p0+p1, blk3 A=CURR+c / c; write WR0_LO+WR0_HI=A (even
    positions approximate, odd exact).  FSM per mode: uop0 entry (reset,
    COUNT(1)->steady), uop1 steady (SUB_DIM_DONE->step), uop2 step (reset,
    COUNT(1)->steady)."""
    _ensure_path()
    from concourse import dve_ops
    from concourse.dve_spec import Spec, Src0, Src1, Bin, Scan, Zero
    from concourse.dve_spec import AluOp as SAluOp
    from concourse.dve_uop import (
        AluInp, AluOp, DelayInp, DveOpSpec, InpSel, OutPath, OutSel,
        Trigger, UopConfig, UopDpConfig, ENABLE,
    )

    NAME = "SEGSUM_MULT_ANT"
    for o in dve_ops.OPS:
        if o.name == NAME:
            return o

    def _ref(in0, in1, c0, c1, c2):
        p = in0.astype(np.float32) * in1.astype(np.float32)
        return np.cumsum(p, axis=-1).astype(np.float32)

    def _dp_regular(step):
        blks = [UopDpConfig() for _ in range(8)]
        blks[0].enable_alu(AluOp.MULTIPLY, AluInp.PREV_ALU_OUT, AluInp.PREV_DELAY_0)
        if step:
            blks[1].enable_alu(AluOp.BYPASS, AluInp.PREV_ALU_OUT)
        else:
            blks[1].enable_alu(AluOp.ADD, AluInp.CURR_ALU_OUT, AluInp.PREV_ALU_OUT)
        for b in range(2, 8):
            blks[b].pass_through_alu()
        return blks

    def _dp_2x(step):
        blks = [UopDpConfig() for _ in range(8)]
        blks[0].enable_alu(AluOp.MULTIPLY, AluInp.PREV_ALU_OUT, AluInp.PREV_DELAY_0)
        blks[0].enable_delay_from_src(DelayInp.PREV_DELAY, 1)  # chain1 <- lane2 (SRC_0_HI)
        blks[0].enable_delay_from_src(DelayInp.PREV_DELAY, 2)  # chain2 <- lane3 (SRC_1_HI)
        blks[1].enable_alu(AluOp.MULTIPLY, AluInp.PREV_DELAY_1, AluInp.PREV_DELAY_2)
        blks[1].enable_delay_from_src(DelayInp.PREV_ALU_OUT, 0)  # chain0 <- p0
        blks[2].enable_alu(AluOp.ADD, AluInp.PREV_ALU_OUT, AluInp.PREV_DELAY_0)
        if step:
            blks[3].enable_alu(AluOp.BYPASS, AluInp.PREV_ALU_OUT)
        else:
            blks[3].enable_alu(AluOp.ADD, AluInp.CURR_ALU_OUT, AluInp.PREV_ALU_OUT)
        for b in range(4, 8):
            blks[b].pass_through_alu()
        return blks

    def _uop(dp, two_x, entry):
        u = UopConfig()
        u.inp[0] = InpSel.SRC_0
        u.inp[1] = InpSel.SRC_1
        u.inp_enable[0] = ENABLE
        u.inp_enable[1] = ENABLE
        if two_x:
            u.inp[2] = InpSel.SRC_0_HI
            u.inp[3] = InpSel.SRC_1_HI
            u.inp_enable[2] = ENABLE
            u.inp_enable[3] = ENABLE
        u.out[OutPath.WR0_LO] = OutSel.ALU_OUT
        u.out_enable[OutPath.WR0_LO] = ENABLE
        if two_x:
            u.out[OutPath.WR0_HI] = OutSel.ALU_OUT
            u.out_enable[OutPath.WR0_HI] = ENABLE
        u.require_inp0 = ENABLE
        u.require_inp1 = ENABLE
        if entry is None:  # steady
            u.trigger = (Trigger.SRC_TENSOR_DONE, Trigger.SUB_DIM_DONE, Trigger.NONE)
            u.next_uop = (0, 2, 0)
        else:  # entry/step: reset pair, one cycle, then steady
            u.trigger = (Trigger.SRC_TENSOR_DONE, Trigger.SUB_DIM_DONE, Trigger.COUNT)
            u.next_uop = (0, 2, 1)
            u.repeat_count = 1
        u.datapath_config = dp
        return u

    uops_1x = [
        _uop(_dp_regular(True), False, True),
        _uop(_dp_regular(False), False, None),
        _uop(_dp_regular(True), False, False),
    ]
    uops_2x = [
        _uop(_dp_2x(True), True, True),
        _uop(_dp_2x(False), True, None),
        _uop(_dp_2x(True), True, False),
    ]

    class _SegsumOp(dve_ops.DveOp):
        def compile(self, ver):
            assert ver == "v3", f"SEGSUM_MULT_ANT authored for v3 only, got {ver}"
            spec = DveOpSpec(
                name=self.name,
                opcode=dve_ops.get_dve_sub_opcode(self.name),
                uops=uops_1x,
                uops_2x=uops_2x,
                perf_max=1,
                rd1_en=True,
            )
            spec.validate(ver)
            return spec

    spec = Spec(
        body=Scan(SAluOp.ADD, Bin(SAluOp.MULTIPLY, Src0, Src1), init=Zero),
        reference=_ref,
    )
    op = _SegsumOp(name=NAME, spec=spec, subdim=True, uops_sha={})
    dve_ops.OPS.append(op)
    dve_ops.CUSTOM_DVE_SPECS[NAME] = spec
    dve_ops._SUB_OPCODE_FOR_NAME[NAME] = (
        dve_ops._CUSTOM_DVE_ROW_BASE + len(dve_ops.OPS) - 1
    )
    return op


def emit_segsum(nc, op, out, in0, in1):
    """SEGSUM_MULT_ANT at 2x: APs [P, S, 8], unit inner stride, 4B-aligned."""
    from concourse import bass_isa, mybir
    from concourse.dve_ops import get_dve_sub_opcode

    v = nc.vector
    if op.name not in nc.m.ant_custom_dve_ops:
        nc.m.ant_custom_dve_ops = sorted({*nc.m.ant_custom_dve_ops, op.name})
    shape = bass_isa.CustomDveShape.STT
    isa_opcode = nc.isa.Opcode[
        f"NEURON_ISA_TPB_OPCODE_CUSTOM_DVE_ANT_{shape.slot()}"
    ].value
    ins = [
        v.lower_ap(in0, for_isa=True, opt=False),
        v.lower_ap(in1, for_isa=True, opt=False),
        mybir.ImmediateValue(dtype=mybir.dt.float32, value=0.0),
        mybir.ImmediateValue(dtype=mybir.dt.float32, value=0.0),
    ]
    outs = [v.lower_ap(out, for_isa=True, opt=False)]
    return v.add_instruction(
        bass_isa.InstCustomDveAnt(
            name=nc.get_next_instruction_name(),
            op_name=op.name,
            rd1_en=True,
            subdim=0x02,
            imm2=0.0,
            shape=shape,
            row=get_dve_sub_opcode(op.name),
            isa_opcode=isa_opcode,
            perf_max=1,
            ins=ins,
            outs=outs,
        )
    )


def build_nc(chunk_cols=None, recip_on_act=True):
    """Per-core Bass program over a packed (128, C*COLS) bf16 shard.

    Chunk j occupies columns [C*off, C*(off+ncol)) of x; per partition the
    chunk block is [q: (x,k)][K: (v,x,k)][V: (v,x)] so the segsum op reads
    unit-stride pages of 8 and q/K/V loads are single contiguous spans."""
    _ensure_path()
    import concourse.tile as tile
    from concourse import bacc, mybir

    f32 = mybir.dt.float32
    bf = mybir.dt.bfloat16
    if chunk_cols is None:
        chunk_cols = CHUNK_COLS
    assert sum(chunk_cols) == COLS

    import bass_rust as _bass_rust
    from concourse.hw_specs import get_activation_tables

    segsum_op = build_segsum_op()

    class _Bacc(bacc.Bacc):
        """Pin Exp to natural_log_exp_and_others (drop `exp` from the
        exp_and_others set) — one ACT_TABLE_LOAD, no per-chunk thrash."""

        def insert_act_table_loads(self):
            has_activation = any(
                isinstance(i, mybir.InstActivation)
                for b in self.main_func.blocks
                for i in b.instructions
            )
            if not has_activation:
                return
            exp_fn = mybir.ActivationFunctionType.Exp
            tables = []
            for name, funcs in get_activation_tables(self.m.arch).items():
                if name == "exp_and_others":
                    funcs = funcs - {exp_fn}
                tables.append((name, funcs))
            _bass_rust.insert_act_table_loads(self, tables)

    nc = _Bacc("TRN2", target_bir_lowering=False, debug=False)
    x = nc.dram_tensor("x", [128, C * COLS], bf, kind="ExternalInput")
    y = nc.dram_tensor("y", [128, NV * COLS], bf, kind="ExternalOutput")

    with tile.TileContext(nc) as tc:
        with (
            tc.tile_pool(name="inp", bufs=2) as in_pool,
            tc.tile_pool(name="work", bufs=2) as work_pool,
            tc.tile_pool(name="pipe", bufs=2) as pipe_pool,
        ):
            offs = []
            o = 0
            for ncol in chunk_cols:
                offs.append(o)
                o += ncol
            state = {}
            v_tiles = {}

            v_loaded = set()

            def emit_v_load(j):
                if j in v_loaded:
                    return
                v_loaded.add(j)
                ncol, off = chunk_cols[j], offs[j]
                base = C * off
                nc.sync.dma_start(
                    out=v_tiles[j],
                    in_=x[:, base + (NK + NK * NV) * ncol:base + C * ncol])

            def emit_head(j):
                """DMA chunk j; 8 segsum dot-product scans; ACT exp."""
                ncol, off = chunk_cols[j], offs[j]
                base = C * off
                q_t = in_pool.tile([128, NK * ncol], bf, name=f"q{j}", tag="q",
                                   bufs=3)
                k_t = in_pool.tile([128, NK * NV * ncol], bf, name=f"k{j}",
                                   tag="k", bufs=3)
                v_t = in_pool.tile([128, NV * ncol], bf, name=f"v{j}",
                                   tag="v", bufs=3)
                v_tiles[j] = v_t
                nc.sync.dma_start(out=q_t, in_=x[:, base:base + NK * ncol])
                kbase = base + NK * ncol
                # K in two halves (v 0-3, v 4-7): the first 4 segsums start
                # on the first half — hides the straggler DMA queue's lag
                half = NK * NV * ncol // 2
                nc.sync.dma_start(out=k_t[:, :half],
                                  in_=x[:, kbase:kbase + half])
                nc.sync.dma_start(out=k_t[:, half:],
                                  in_=x[:, kbase + half:kbase + NK * NV * ncol])
                # V of the PREVIOUS chunk loads after this chunk's K (V is not
                # read until tailA(j-1), one pipeline stage later)
                if j > 0:
                    emit_v_load(j - 1)
                if j == len(chunk_cols) - 1:
                    emit_v_load(j)

                # qk[v, x] = segsum(K[v, x, :] * q[x, :])[..., 7]
                prod = work_pool.tile([128, NK * NV * ncol], bf,
                                      name=f"prod{j}", tag="prod")
                p4 = prod.rearrange("p (v s n) -> p v s n", v=NV, n=NK)
                k4 = k_t.rearrange("p (v s n) -> p v s n", v=NV, n=NK)
                q3 = q_t.rearrange("p (s n) -> p s n", n=NK)
                for vv in range(NV):
                    emit_segsum(nc, segsum_op, p4[:, vv], k4[:, vv], q3)

                # e = exp(qk / sqrt(NK)) on ScalarE, reading page-final
                # elements at stride 8 (exact positions of the 2x scan)
                e = pipe_pool.tile([128, NV * ncol], bf, name=f"e{j}", tag="e",
                                   bufs=3)
                qk = p4[:, :, :, NK - 1]
                e3 = e.rearrange("p (v s) -> p v s", v=NV)
                nc.scalar.activation(e3, qk,
                                     mybir.ActivationFunctionType.Exp,
                                     scale=_SCALE)
                state[j] = (e, v_t)

            def emit_tail_a(j):
                """Chunk j softmax part 1: denom pair-tree, r on ACT, e*V."""
                ncol, off = chunk_cols[j], offs[j]
                e, v_t = state[j]
                t1 = pipe_pool.tile([128, 4 * ncol], bf, name=f"t1{j}", tag="t1")
                with nc.allow_low_precision(reason="softmax denom in bf16"):
                    nc.vector.tensor_tensor(t1, e[:, 0:4 * ncol], e[:, 4 * ncol:],
                                            mybir.AluOpType.add)
                    nc.vector.tensor_tensor(t1[:, 0:2 * ncol], t1[:, 0:2 * ncol],
                                            t1[:, 2 * ncol:4 * ncol],
                                            mybir.AluOpType.add)
                    s = t1[:, 2 * ncol:3 * ncol]
                    nc.vector.tensor_tensor(s, t1[:, 0:ncol],
                                            t1[:, ncol:2 * ncol],
                                            mybir.AluOpType.add)
                r = pipe_pool.tile([128, ncol], bf, name=f"r{j}", tag="r")
                if recip_on_act:
                    # r = exp(-ln s) on ScalarE: off the DVE critical path,
                    # single act-table set (ln+exp coexist)
                    ls = pipe_pool.tile([128, ncol], f32, name=f"ls{j}", tag="ls")
                    nc.scalar.activation(ls, s, mybir.ActivationFunctionType.Ln)
                    nc.scalar.activation(r, ls, mybir.ActivationFunctionType.Exp,
                                         scale=-1.0)
                else:
                    rf = pipe_pool.tile([128, ncol], f32, name=f"rf{j}", tag="ls")
                    nc.vector.reciprocal(rf, s)
                    with nc.allow_low_precision(reason="r in bf16"):
                        nc.vector.tensor_copy(r, rf)

                # e *= V (does not depend on r; covers the ACT latency)
                e3 = e.rearrange("p (v x) -> p v x", v=NV)
                v3 = v_t.rearrange("p (v x) -> p v x", v=NV)
                nc.vector.tensor_tensor(e3, e3, v3, mybir.AluOpType.mult)
                state[j] = (e, r)

            def emit_tail_b(j):
                """Chunk j softmax part 2: multiply by r, store on the
                scalar ring."""
                ncol, off = chunk_cols[j], offs[j]
                e, r = state.pop(j)
                e3 = e.rearrange("p (v x) -> p v x", v=NV)
                r_b = r.unsqueeze(1).broadcast_to((128, NV, ncol))
                nc.vector.tensor_tensor(e3, r_b, e3, mybir.AluOpType.mult)
                nc.scalar.dma_start(out=y[:, NV * off:NV * (off + ncol)], in_=e)

            # software pipeline, two-chunk skew on the r-multiply:
            #   head(0), head(1), tailA(0), head(2), tailB(0), tailA(1), ...
            # tailA(n-2) is emitted BEFORE head(n-1): its denoms/ACT round
            # trip runs while the last chunk's K is still streaming in, so
            # the post-DMA drain is only the last chunk's own chain.
            n = len(chunk_cols)
            for j in range(n):
                if j == n - 1:
                    emit_v_load(j - 1)
                    emit_tail_a(j - 1)
                emit_head(j)
                if j >= 2:
                    emit_tail_b(j - 2)
                if j >= 1 and j != n - 1:
                    emit_tail_a(j - 1)
            emit_tail_b(n - 2)
            emit_tail_a(n - 1)
            emit_tail_b(n - 1)
    nc.compile()
    return nc


_NC_CACHE = {}

BUILD_CFG = {
    "chunk_cols": CHUNK_COLS,
}


def _get_nc(**cfg):
    cfg = {**BUILD_CFG, **cfg}
    key = tuple(sorted(
        (k, tuple(v) if isinstance(v, list) else v) for k, v in cfg.items()
    ))
    if key not in _NC_CACHE:
        _NC_CACHE[key] = build_nc(**cfg)
    return _NC_CACHE[key]


def make_in_maps(inp, chunk_cols):
    """Shard + pack: core i gets batch i//2, H-half i%2, as a bf16
    (128, C*COLS) array laid out [p][chunk][q:(x,k)][K:(v,x,k)][V:(v,x)]."""
    in_maps = []
    for core in range(N_CORES):
        b, half = core // 2, core % 2
        shard = np.asarray(
            inp[b, :, half * ROWS:(half + 1) * ROWS, :], dtype=np.float32
        ).reshape(C, 128, COLS).astype(BF16)
        q_all = shard[:NK]                                   # (k, p, col)
        k_all = shard[NK:NK + NK * NV].reshape(NK, NV, 128, COLS)
        v_all = shard[NK + NK * NV:]                         # (v, p, col)
        blocks = []
        off = 0
        for ncol in chunk_cols:
            sl = slice(off, off + ncol)
            qb = np.ascontiguousarray(
                q_all[:, :, sl].transpose(1, 2, 0)).reshape(128, NK * ncol)
            kb = np.ascontiguousarray(
                k_all[:, :, :, sl].transpose(2, 1, 3, 0)).reshape(
                    128, NK * NV * ncol)
            vb = np.ascontiguousarray(
                v_all[:, :, sl].transpose(1, 0, 2)).reshape(128, NV * ncol)
            blocks.append(np.concatenate([qb, kb, vb], axis=1))
            off += ncol
        in_maps.append({"x": np.concatenate(blocks, axis=1)})
    return in_maps


def assemble_out(results, chunk_cols):
    out = np.empty((B, NV, H, W), np.float32)
    for core in range(N_CORES):
        b, half = core // 2, core % 2
        yb = results[core]["y"]  # (128, NV*COLS) bf16, chunk-major
        blocks = []
        off = 0
        for ncol in chunk_cols:
            blocks.append(
                yb[:, NV * off:NV * (off + ncol)].reshape(128, NV, ncol))
            off += ncol
        full = np.concatenate(blocks, axis=2)            # (128, NV, COLS)
        out[b, :, half * ROWS:(half + 1) * ROWS, :] = (
            full.transpose(1, 0, 2).astype(np.float32).reshape(NV, ROWS, W))
    return out


def run_spmd(inp, trace=False, build_cfg=None, **kwargs):
    """Run the SPMD kernel on 8 cores; returns (full_output, BassKernelResults)."""
    _ensure_path()
    from concourse.bass_utils import run_bass_kernel_spmd

    inp = np.asarray(inp)
    assert inp.shape == (B, C, H, W), inp.shape
    cfg = {**BUILD_CFG, **(build_cfg or {})}
    nc = _get_nc(**cfg)
    res = run_bass_kernel_spmd(
        nc, make_in_maps(inp, cfg["chunk_cols"]), list(range(N_CORES)),
        trace=trace, **kwargs
    )
    return assemble_out(res.results, cfg["chunk_cols"]), res


def kernel(inp):
    out, _ = run_spmd(inp, trace=False)
    return out
